# revision 27
# baseline (speedup 1.0000x reference)
"""Trainium2 Bass kernel for nn_CausalSelfAttention_56925496541402.

Sliding-window (1024) causal self-attention with rotary embedding,
rms-norm on q/k, and a value-embedding (VE) sigmoid gate. B=1, T=4096,
8 heads x 128 head_dim, n_embd=1024.

Sharding: one head per NeuronCore (8 cores). Each core computes its
head's q/k/v projections, rope+rmsnorm, windowed attention, and its
head's slice of the output projection; the host sums the 8 partial
[4096,1024] outputs (row-block contraction of c_proj).

Layouts per core (SBUF):
  qT/kT: [head_dim=128 part, T free]; rope via partition-shifted DVE
         operands; rms partition-sum via all-ones matmul; rsqrt via the
         Abs_reciprocal_sqrt ACT table.
  gate:  [t-chunk part, 1] columns (32 one-column matmuls off a packed
         x32 tile + ONE sigmoid); applied post-transpose as a
         per-partition scalar fused with the psum drain
         (vsl = (veT*g) + v^T in one scalar_tensor_tensor).
  v:     [T part (128-chunks), head_dim free] (PE-transposed).
  S^T:   [j=key part, i=query free]; chunk PAIRS share one 2-bank psum
         tile and ONE exp; softmax denominator = DVE fold of the exp'd
         tiles into a [128,512] f16 slab + a single all-ones matmul;
         masks multiplicative post-exp.

DMA order: x32 gate slice (0.8us), weights, first x chunk split in
halves, then x chunks interleaved with cc/ss/vet slices; masks and
w_proj after the x stream.

fp16 data path; matmul accumulation and softmax stats in f32.
exp(S*scale - 4) keeps attention weights inside fp16 range. ACT table
sets: sigmoid -> abs_reciprocal_sqrt_and_small -> exp (3 loads total).
"""
import sys
sys.path.insert(0, "/opt/trn_rl_repo")
import math
import numpy as np

T = 4096
TB = 512           # t-block width
NBLK = T // TB
D = 128            # head dim
C = 1024           # n_embd
NCO = C // 128     # embed chunks
WIN = 1024
NCORES = 8
SCALE = 1.0 / math.sqrt(D)
EXP_BIAS = -4.0    # exp(S*scale - 4): fp16-safe range, cancels in normalize

_prog_cache = {}
_last_in_maps = None


def _chunk_list(b):
    """Key chunks for query block b (i0=512b): (j0, mask_idx, lo, hi).

    [lo, hi) is the computed query range (the chunk's visible window);
    the mask multiply is applied on the 128-wide triangle boundary
    [mlo, mlo+128) inside it. The first chunk covers [0, 512) so its
    start=True matmul initializes every psum column.
    mask m<4 : low window edge, visible iff ii < jj + 128*m
    mask m>=4: causal edge,     visible iff ii >= jj + 128*(m-4)
    """
    i0 = TB * b
    out = []
    for c in range(4):           # full chunks (emitted first)
        j0 = i0 - 512 + 128 * c
        if j0 >= 0:
            out.append((j0, None, 0, 512))
    for c in range(4):           # causal chunks: visible i in [128c, 512)
        j0 = i0 + 128 * c
        out.append((j0, 4 + c, 128 * c, 512))
    for c in range(4):           # low-edge chunks: visible i in [0, 128c+128)
        j0 = i0 - 1024 + 128 * c
        if j0 >= 0:
            out.append((j0, c, 0, 128 * (c + 1)))
    if b == 0:
        # no full chunks: widest causal chunk (c=0, [0,512)) is already first
        assert out[0][2] == 0 and out[0][3] == 512
    return out


def _pair_list(b):
    """Chunk pairs for query block b. Each pair is a list of 1-2 chunk
    records (j0, mi, lo, hi, off) sharing one [128,1024] psum tile (two
    banks) and one exp; off is the chunk's column offset in the tile.

    A matmul output cannot cross the 512-f32 psum bank boundary, so
    chunks are paired largest-with-smallest: the second chunk sits at
    off=512 (bank 1), or densely at off=w0 when both fit in bank 0.
    Pairing keeps the exp'd range [0, off1+w1) contiguous (no stale-psum
    gap), and the widest chunk of pair 0 still covers queries [0,512) so
    the first PV matmul's start=True initializes every psum column."""
    chunks = sorted(_chunk_list(b), key=lambda c: c[3] - c[2], reverse=True)
    pairs = []
    n = len(chunks)
    for i in range(n // 2):
        a, z = chunks[i], chunks[n // 2 + i]
        wa = a[3] - a[2]
        off1 = wa if wa + (z[3] - z[2]) <= 512 else 512
        assert off1 == wa or wa == 512
        pairs.append([(a[0], a[1], a[2], a[3], 0),
                      (z[0], z[1], z[2], z[3], off1)])
    if n % 2:
        m = chunks[n // 2]
        pairs.append([(m[0], m[1], m[2], m[3], 0)])
    assert pairs[0][0][2] == 0 and pairs[0][0][3] == 512
    return pairs


def _build_program(nreps=1):
    import concourse.bass as bass
    import concourse.mybir as mybir
    import concourse.tile as tile
    from concourse import bacc
    from concourse.masks import make_identity

    F32 = mybir.dt.float32
    F16 = mybir.dt.float16
    AF = mybir.ActivationFunctionType
    MUL = mybir.AluOpType.mult
    ADD = mybir.AluOpType.add
    ts = bass.ts

    nc = bacc.Bacc("TRN2", target_bir_lowering=False, debug=False,
                   enable_asserts=True, num_devices=1)

    # x_pre[p, co*T + t] = x[t, co*128+p]: per-partition contiguous lines
    xT = nc.dram_tensor("xT", [128, NCO * T], F16, kind="ExternalInput").ap()
    x32_d = nc.dram_tensor("x32", [32, T], F16, kind="ExternalInput").ap()
    cc_d = nc.dram_tensor("cc", [D, T], F16, kind="ExternalInput").ap()
    ss_d = nc.dram_tensor("ssw", [D, T], F16, kind="ExternalInput").ap()
    # veT2[p, ch*128 + dd] = 2*ve[128ch+p, head_slice+dd]
    veT_d = nc.dram_tensor("veT", [D, T], F16, kind="ExternalInput").ap()
    # w_pre[p, co*128 + d] = w[co*128+p, d]: contiguous per-partition lines
    wq_d = nc.dram_tensor("wq", [128, C], F16, kind="ExternalInput").ap()
    wk_d = nc.dram_tensor("wk", [128, C], F16, kind="ExternalInput").ap()
    wv_d = nc.dram_tensor("wv", [128, C], F16, kind="ExternalInput").ap()
    wp_d = nc.dram_tensor("wp", [D, C], F16, kind="ExternalInput").ap()
    wg_d = nc.dram_tensor("wg", [32, 1], F16, kind="ExternalInput").ap()
    # masks pre-arranged host-side to [p, m, i]
    mk_d = nc.dram_tensor("masks", [128, 8 * 512], F16, kind="ExternalInput").ap()
    on_d = nc.dram_tensor("ones", [128, 128], F16, kind="ExternalInput").ap()
    out_d = nc.dram_tensor("out", [T, C], F16, kind="ExternalOutput").ap()

    xT3 = xT.rearrange("p (co t) -> p co t", co=NCO)

    with tile.TileContext(nc) as tc:
        with tc.tile_pool(name="const", bufs=1) as cst:
            x32 = cst.tile([32, T], F16, tag="x32")
            w_sbs = []
            for wd, nm in ((wq_d, "wq"), (wk_d, "wk"), (wv_d, "wv")):
                w_sb = cst.tile([128, NCO, D], F16, tag=f"w{nm}")
                nc.sync.dma_start(w_sb[:], wd.rearrange("p (co d) -> p co d",
                                                        co=NCO))
                w_sbs.append(w_sb)
            wq_sb, wk_sb, wv_sb = w_sbs
            wg_sb = cst.tile([32, 1], F16, tag="wg")
            nc.sync.dma_start(wg_sb[:], wg_d)
            wp_sb = cst.tile([128, C], F16, tag="wp")
            mk_sb = cst.tile([128, 8, 512], F16, tag="mk")
            on_sb = cst.tile([128, 128], F16, tag="on")
            nc.sync.dma_start(on_sb[:], on_d)
            ident = cst.tile([128, 128], F16, tag="ident")
            make_identity(nc, ident[:])
            eps = cst.tile([128, 1], F32, tag="eps")
            nc.gpsimd.memset(eps[:], 1e-6)
            eb = cst.tile([128, 1], F32, tag="eb")
            nc.gpsimd.memset(eb[:], EXP_BIAS)
            gcol = cst.tile([128, 32], F16, tag="gcol")
            qTn = cst.tile([128, T], F16, tag="qTn")
            kTn = cst.tile([128, T], F16, tag="kTn")
            vsl = cst.tile([128, T // 128, D], F16, tag="vsl")
            cc_sb = cst.tile([128, T], F16, tag="cc")
            ss_sb = cst.tile([128, T], F16, tag="ssw")
            vet = cst.tile([128, 32, 128], F16, tag="ve")

            for _rep in range(nreps):
                # ---- phase 1: VE gate columns + q/k/v projections,
                # rope+rmsnorm, v transpose+gate. sumsq matmuls are emitted
                # after all three projections so the PE never stalls on the
                # DVE rope chain.
                with tc.tile_pool(name="xp", bufs=3) as xp, \
                     tc.tile_pool(name="sc1", bufs=4) as sc, \
                     tc.tile_pool(name="pps", bufs=3, space="PSUM") as pps, \
                     tc.tile_pool(name="gps", bufs=1, space="PSUM") as gps, \
                     tc.tile_pool(name="sqps", bufs=2, space="PSUM") as sqps, \
                     tc.tile_pool(name="tps", bufs=2, space="PSUM") as tps:
                    x_sb2 = None
                    for tb in range(NBLK):
                        sl = ts(tb, TB)
                        if tb % 2 == 0:
                            x_sb2 = xp.tile([128, NCO, 2 * TB], F16, tag="x")
                            vet3 = veT_d.rearrange("p (ch dd) -> p ch dd",
                                                   ch=32)
                            sl2 = ts(tb // 2, 2 * TB)
                            if tb == 0:
                                # split first chunk: block-0 projections
                                # start after a half-size transfer; the
                                # gate slice and rope tables slot between
                                nc.sync.dma_start(
                                    x_sb2[:, :, 0:TB], xT3[:, :, 0:TB])
                                if _rep == 0:
                                    nc.sync.dma_start(x32[:], x32_d)
                                    nc.sync.dma_start(cc_sb[:, sl2],
                                                      cc_d[:, sl2])
                                    nc.sync.dma_start(ss_sb[:, sl2],
                                                      ss_d[:, sl2])
                                nc.sync.dma_start(
                                    x_sb2[:, :, TB:2 * TB],
                                    xT3[:, :, TB:2 * TB])
                                if _rep == 0:
                                    nc.sync.dma_start(
                                        vet[:, 0:8, :], vet3[:, 0:8, :])
                            else:
                                nc.sync.dma_start(
                                    x_sb2[:], xT3[:, :, ts(tb // 2, 2 * TB)])
                                if _rep == 0:
                                    nc.sync.dma_start(cc_sb[:, sl2],
                                                      cc_d[:, sl2])
                                    nc.sync.dma_start(ss_sb[:, sl2],
                                                      ss_d[:, sl2])
                                    nc.sync.dma_start(
                                        vet[:, 4 * tb:4 * tb + 8, :],
                                        vet3[:, 4 * tb:4 * tb + 8, :])
                                    if tb == NBLK - 2:
                                        nc.sync.dma_start(wp_sb[:], wp_d)
                                        nc.sync.dma_start(
                                            mk_sb[:],
                                            mk_d.rearrange(
                                                "p (m i) -> p m i", m=8))
                        x_sb = x_sb2[:, :, ts(tb % 2, TB)]
                        sq_tiles = []
                        for w_sb in (wq_sb, wk_sb):
                            up = pps.tile([128, TB], F32, tag="proj")
                            for co in range(NCO):
                                nc.tensor.matmul(up[:], w_sb[:, co, :],
                                                 x_sb[:, co, :],
                                                 start=(co == 0), stop=(co == NCO - 1))
                            u16 = sc.tile([128, TB], F16, tag="u16")
                            nc.scalar.copy(u16[:], up[:])
                            # rope: p[d] = u[d]*ssw[d]; y = u*cc + swap64(p)
                            t1 = sc.tile([128, TB], F16, tag="t1")
                            nc.vector.tensor_tensor(t1[:], u16[:], cc_sb[:, sl], MUL)
                            p = sc.tile([128, TB], F16, tag="p")
                            nc.vector.tensor_tensor(p[:], u16[:], ss_sb[:, sl], MUL)
                            pr = sc.tile([128, TB], F16, tag="pr")
                            nc.vector.tensor_copy(pr[0:64, :], p[64:128, :])
                            nc.vector.tensor_copy(pr[64:128, :], p[0:64, :])
                            y = sc.tile([128, TB], F16, tag="y")
                            nc.vector.tensor_tensor(y[:], t1[:], pr[:], ADD)
                            sq = sc.tile([128, TB], F16, tag="sq")
                            nc.vector.tensor_tensor(sq[:], y[:], y[:], MUL)
                            sq_tiles.append((sq, y))
                        # v projection; gate applied post-transpose
                        vp = pps.tile([128, TB], F32, tag="proj")
                        for co in range(NCO):
                            nc.tensor.matmul(vp[:], wv_sb[:, co, :], x_sb[:, co, :],
                                             start=(co == 0), stop=(co == NCO - 1))
                        v16 = sc.tile([128, TB], F16, tag="v16")
                        nc.scalar.copy(v16[:], vp[:])
                        if tb == 0:
                            # gate cols: gcol[p,c] = sigmoid(x[128c+p,:32]@wg)
                            # emitted here so the PE has projection work
                            # first and the ACT sigmoid precedes all rsqrts
                            gp = gps.tile([128, 32], F32, tag="gps")
                            for cp in range(32):
                                nc.tensor.matmul(
                                    gp[:, cp:cp + 1],
                                    x32[:, ts(cp, 128)], wg_sb[:],
                                    start=True, stop=True)
                            nc.scalar.activation(gcol[:], gp[:], AF.Sigmoid)
                        # rms-norm scale (partition-axis sum via all-ones matmul)
                        for (sq, y), slab in zip(sq_tiles, (qTn, kTn)):
                            sp = sqps.tile([128, TB], F32, tag="sumsq")
                            nc.tensor.matmul(sp[:], on_sb[:], sq[:],
                                             start=True, stop=True)
                            rs = sc.tile([128, TB], F16, tag="rs")
                            nc.scalar.activation(rs[:], sp[:],
                                                 AF.Abs_reciprocal_sqrt,
                                                 scale=1.0 / D, bias=eps[:])
                            nc.vector.tensor_tensor(slab[:, sl], y[:], rs[:], MUL)
                        # v^T chunks + VE gate: vsl = (veT*g) + v^T
                        for kk in range(4):
                            ch = 4 * tb + kk
                            tp = tps.tile([128, 128], F16, tag="tp")
                            nc.tensor.transpose(tp[:], v16[:, ts(kk, 128)], ident[:])
                            nc.vector.scalar_tensor_tensor(
                                vsl[:, ch, :], vet[:, ch, :],
                                gcol[:, ch:ch + 1], tp[:], MUL, ADD)

                # ---- phase 2: windowed attention + output projection ----
                # ACT funcs here: Exp only. Chunk pairs share one psum tile
                # and one exp. The S matmuls for pair p+LAG are emitted
                # before the PV of pair p so the PE never stalls on the ACT
                # exp chain.
                with tc.tile_pool(name="ptp", bufs=6) as ptp, \
                     tc.tile_pool(name="acp", bufs=2) as acp, \
                     tc.tile_pool(name="sc2", bufs=4) as sc2, \
                     tc.tile_pool(name="outp", bufs=4) as outp, \
                     tc.tile_pool(name="sps", bufs=2, space="PSUM") as sps, \
                     tc.tile_pool(name="yps", bufs=2, space="PSUM") as yps, \
                     tc.tile_pool(name="dps", bufs=1, space="PSUM") as dps, \
                     tc.tile_pool(name="ops", bufs=1, space="PSUM") as ops:
                    # dummy exp on a tiny tile: forces the exp table load
                    # onto the ACT queue early, overlapping phase-1 tail
                    warm = sc2.tile([128, 1], F16, tag="warm")
                    nc.scalar.activation(warm[:], eb[:], AF.Exp)

                    def emit_outproj(yt, i0, tcc):
                        ost = outp.tile([128, 1024], F16, tag="ost")
                        for hh in range(2):
                            op2 = ops.tile([128, 512], F32, tag="opair")
                            nc.tensor.matmul(op2[:],
                                             yt[:, ts(tcc, 128)],
                                             wp_sb[:, ts(hh, 512)],
                                             start=True, stop=True)
                            (nc.scalar.copy if hh == 0 else
                             nc.vector.tensor_copy)(ost[:, ts(hh, 512)],
                                                    op2[:])
                        nc.sync.dma_start(
                            out_d[i0 + 128 * tcc:i0 + 128 * (tcc + 1), :],
                            ost[:])

                    def emit_tail(yp, acc):
                        """denominator -> reciprocal -> normalized yt"""
                        dp = dps.tile([128, TB], F32, tag="d")
                        nc.tensor.matmul(dp[:], on_sb[:], acc[:],
                                         start=True, stop=True)
                        rc = sc2.tile([128, TB], F32, tag="rc")
                        nc.vector.reciprocal_approx_fast(rc[:], dp[:])
                        yt = sc2.tile([128, TB], F16, tag="yt")
                        nc.vector.tensor_tensor(yt[:], yp[:], rc[:], MUL)
                        return yt

                    LAG = 1   # pairs of lookahead between exp and PV
                    prev = None   # (yp, acc, i0) of previous block
                    pending_out = None
                    for b in range(NBLK):
                        i0 = TB * b
                        pairs = _pair_list(b)
                        n = len(pairs)
                        yp = yps.tile([128, TB], F32, tag="y")
                        acc = acp.tile([128, TB], F16, tag="acc")
                        pts = {}
                        nch = sum(len(pr) for pr in pairs)
                        ich = 0   # chunk counter for PV start/stop flags
                        for step in range(n + LAG):
                            if step < n:
                                pair = pairs[step]
                                wtot = sum(hi - lo for (_, _, lo, hi, _) in pair)
                                sp2 = sps.tile([128, 1024], F32, tag="spair")
                                for (j0, mi, lo, hi, off) in pair:
                                    nc.tensor.matmul(
                                        sp2[:, off:off + hi - lo],
                                        kTn[:, j0:j0 + 128],
                                        qTn[:, i0 + lo:i0 + hi],
                                        start=True, stop=True)
                                pt = ptp.tile([128, 1024], F16, tag="pt")
                                nc.scalar.activation(pt[:, 0:wtot],
                                                     sp2[:, 0:wtot],
                                                     AF.Exp, scale=SCALE,
                                                     bias=eb[:])
                                for (j0, mi, lo, hi, off) in pair:
                                    if mi is not None:
                                        mlo = 128 * (mi if mi < 4 else mi - 4)
                                        psl = pt[:, off + mlo - lo:
                                                 off + mlo - lo + 128]
                                        nc.vector.tensor_tensor(
                                            psl, psl,
                                            mk_sb[:, mi, mlo:mlo + 128], MUL)
                                # denominator fold (f16; <=12 summands/elem)
                                for (j0, mi, lo, hi, off) in pair:
                                    src = pt[:, off:off + hi - lo]
                                    if step == 0 and off == 0:
                                        nc.vector.tensor_copy(acc[:], src)
                                    else:
                                        nc.vector.tensor_tensor(
                                            acc[:, lo:hi], acc[:, lo:hi],
                                            src, ADD)
                                pts[step] = pt
                            if step == 1 and prev is not None:
                                pyp, pacc, pi0 = prev
                                pending_out = (emit_tail(pyp, pacc), pi0)
                                prev = None
                            if step == 3 and pending_out is not None:
                                for tcc in range(4):
                                    emit_outproj(*pending_out, tcc)
                                pending_out = None
                            idx = step - LAG
                            if 0 <= idx < n:
                                pt = pts.pop(idx)
                                for (j0, mi, lo, hi, off) in pairs[idx]:
                                    st, sp_ = (ich == 0), (ich == nch - 1)
                                    nc.tensor.matmul(
                                        yp[:, lo:hi],
                                        vsl[:, j0 // 128, :],
                                        pt[:, off:off + hi - lo],
                                        start=st, stop=sp_)
                                    ich += 1
                        prev = (yp, acc, i0)
                    # final block tail + outproj
                    pyt = emit_tail(prev[0], prev[1])
                    for tcc in range(4):
                        emit_outproj(pyt, prev[2], tcc)

    nc.finalize()
    return nc


def _w_pre(w):
    # w_pre[p, co*128 + d] = w[co*128+p, d]
    return np.ascontiguousarray(
        w.reshape(NCO, 128, D).transpose(1, 0, 2).reshape(128, C)
    ).astype(np.float16)


def _build_masks():
    jj = np.arange(128)[:, None]
    ii = np.arange(512)[None, :]
    mk = np.zeros((8, 128, 512), dtype=np.float16)
    for m in range(4):
        mk[m] = (ii < jj + 128 * m).astype(np.float16)
    for m in range(4):
        mk[4 + m] = (ii >= jj + 128 * m).astype(np.float16)
    # host pre-arrange to [p, m, i] so the DMA is a straight copy
    return np.ascontiguousarray(mk.transpose(1, 0, 2).reshape(128, 8 * 512))


def kernel(x, ve, cos, sin, wq, wk, wv, w_gate, w_proj, window_size):
    from concourse.bass_utils import run_bass_kernel_spmd

    assert int(np.asarray(window_size)) == WIN
    x = np.asarray(x, dtype=np.float32)
    ve = np.asarray(ve, dtype=np.float32)
    cos = np.asarray(cos, dtype=np.float32).reshape(T, 64)
    sin = np.asarray(sin, dtype=np.float32).reshape(T, 64)
    wq = np.asarray(wq, dtype=np.float32)
    wk = np.asarray(wk, dtype=np.float32)
    wv = np.asarray(wv, dtype=np.float32)
    w_gate = np.asarray(w_gate, dtype=np.float32)
    w_proj = np.asarray(w_proj, dtype=np.float32)
    assert x.shape == (1, T, C) and ve.shape == (1, T, C)

    if "nc" not in _prog_cache:
        _prog_cache["nc"] = _build_program()
    nc = _prog_cache["nc"]

    # x_pre[p, co*T + t] = x[t, co*128+p]
    xT_h = np.ascontiguousarray(
        x[0].T.reshape(NCO, 128, T).transpose(1, 0, 2).reshape(128, NCO * T)
    ).astype(np.float16)
    x32_h = np.ascontiguousarray(x[0][:, 0:32].T).astype(np.float16)
    cosT, sinT = cos.T, sin.T                                # [64, T]
    cc = np.concatenate([cosT, cosT], axis=0).astype(np.float16)
    # p[d] = u[d]*ssw[d]; y[d] = u[d]*cc[d] + p[swap(d)]
    # => ssw = [-sinT; sinT]
    ssw = np.concatenate([-sinT, sinT], axis=0).astype(np.float16)
    masks = _build_masks()

    in_maps = []
    for h in range(NCORES):
        d = D * h
        # veT2[p, 128ch+dd] = 2*ve[128ch+p, d+dd]
        ve2 = np.ascontiguousarray(
            (2.0 * ve[0][:, d:d + D]).reshape(32, 128, 128)
            .transpose(1, 0, 2).reshape(128, T)).astype(np.float16)
        in_maps.append({
            "xT": xT_h,
            "x32": x32_h,
            "cc": cc,
            "ssw": ssw,
            "veT": ve2,
            "wq": _w_pre(wq[:, d:d + D]),
            "wk": _w_pre(wk[:, d:d + D]),
            "wv": _w_pre(wv[:, d:d + D]),
            "wp": np.ascontiguousarray(w_proj[d:d + D, :]).astype(np.float16),
            "wg": w_gate[:, h:h + 1].astype(np.float16),
            "masks": masks,
            "ones": np.ones((128, 128), dtype=np.float16),
        })

    global _last_in_maps
    _last_in_maps = in_maps
    res = run_bass_kernel_spmd(nc, in_maps, core_ids=list(range(NCORES)))
    out = np.zeros((T, C), dtype=np.float32)
    for h in range(NCORES):
        out += res.results[h]["out"].astype(np.float32)
    return out.reshape(1, T, C)


# revision 32
# speedup vs baseline: 1.2634x; 1.2634x over previous
"""Trainium2 Bass kernel for nn_CausalSelfAttention_56925496541402.

Sliding-window (1024) causal self-attention with rotary embedding,
rms-norm on q/k, and a value-embedding (VE) sigmoid gate. B=1, T=4096,
8 heads x 128 head_dim, n_embd=1024.

Sharding: one head per NeuronCore (8 cores). Each core computes its
head's q/k/v projections, rope+rmsnorm, windowed attention, and its
head's slice of the output projection; the host sums the 8 partial
[4096,1024] outputs (row-block contraction of c_proj).

Layouts per core (SBUF):
  qT/kT: [head_dim=128 part, T free]; rope via partition-shifted DVE
         operands; rms partition-sum via all-ones matmul; rsqrt via the
         Abs_reciprocal_sqrt ACT table.
  gate:  [t-chunk part, 1] columns (32 one-column matmuls off a packed
         x32 tile + ONE sigmoid); applied post-transpose as a
         per-partition scalar fused with the psum drain
         (vsl = (veT*g) + v^T in one scalar_tensor_tensor).
  v:     [T part (128-chunks), head_dim free] (PE-transposed).
  S^T:   [j=key part, i=query free]; chunk PAIRS share one 2-bank psum
         tile and ONE exp; softmax denominator = DVE fold of the exp'd
         tiles into a [128,512] f16 slab + a single all-ones matmul;
         masks multiplicative post-exp.

DMA order: x32 gate slice (0.8us), weights, first x chunk split in
halves, then x chunks interleaved with cc/ss/vet slices; masks and
w_proj after the x stream.

fp16 data path; matmul accumulation and softmax stats in f32.
exp(S*scale - 4) keeps attention weights inside fp16 range. ACT table
sets: sigmoid -> abs_reciprocal_sqrt_and_small -> exp (3 loads total).
"""
import sys
sys.path.insert(0, "/opt/trn_rl_repo")
import math
import numpy as np

T = 4096
TB = 512           # t-block width
NBLK = T // TB
D = 128            # head dim
C = 1024           # n_embd
NCO = C // 128     # embed chunks
WIN = 1024
NCORES = 8
SCALE = 1.0 / math.sqrt(D)
EXP_BIAS = -4.0    # exp(S*scale - 4): fp16-safe range, cancels in normalize

_prog_cache = {}
_last_in_maps = None


def _chunk_list(b):
    """Key chunks for query block b (i0=512b): (j0, mask_idx, lo, hi).

    [lo, hi) is the computed query range (the chunk's visible window);
    the mask multiply is applied on the 128-wide triangle boundary
    [mlo, mlo+128) inside it. The first chunk covers [0, 512) so its
    start=True matmul initializes every psum column.
    mask m<4 : low window edge, visible iff ii < jj + 128*m
    mask m>=4: causal edge,     visible iff ii >= jj + 128*(m-4)
    """
    i0 = TB * b
    out = []
    for c in range(4):           # full chunks (emitted first)
        j0 = i0 - 512 + 128 * c
        if j0 >= 0:
            out.append((j0, None, 0, 512))
    for c in range(4):           # causal chunks: visible i in [128c, 512)
        j0 = i0 + 128 * c
        out.append((j0, 4 + c, 128 * c, 512))
    for c in range(4):           # low-edge chunks: visible i in [0, 128c+128)
        j0 = i0 - 1024 + 128 * c
        if j0 >= 0:
            out.append((j0, c, 0, 128 * (c + 1)))
    if b == 0:
        # no full chunks: widest causal chunk (c=0, [0,512)) is already first
        assert out[0][2] == 0 and out[0][3] == 512
    return out


def _pair_list(b):
    """Chunk pairs for query block b. Each pair is a list of 1-2 chunk
    records (j0, mi, lo, hi, off) sharing one [128,1024] psum tile (two
    banks) and one exp; off is the chunk's column offset in the tile.

    A matmul output cannot cross the 512-f32 psum bank boundary, so
    chunks are paired largest-with-smallest: the second chunk sits at
    off=512 (bank 1), or densely at off=w0 when both fit in bank 0.
    Pairing keeps the exp'd range [0, off1+w1) contiguous (no stale-psum
    gap), and the widest chunk of pair 0 still covers queries [0,512) so
    the first PV matmul's start=True initializes every psum column."""
    chunks = sorted(_chunk_list(b), key=lambda c: c[3] - c[2], reverse=True)
    pairs = []
    n = len(chunks)
    for i in range(n // 2):
        a, z = chunks[i], chunks[n // 2 + i]
        wa = a[3] - a[2]
        off1 = wa if wa + (z[3] - z[2]) <= 512 else 512
        assert off1 == wa or wa == 512
        pairs.append([(a[0], a[1], a[2], a[3], 0),
                      (z[0], z[1], z[2], z[3], off1)])
    if n % 2:
        m = chunks[n // 2]
        pairs.append([(m[0], m[1], m[2], m[3], 0)])
    assert pairs[0][0][2] == 0 and pairs[0][0][3] == 512
    return pairs


def _build_program(nreps=1):
    import concourse.bass as bass
    import concourse.mybir as mybir
    import concourse.tile as tile
    from concourse import bacc
    from concourse.masks import make_identity

    F32 = mybir.dt.float32
    F16 = mybir.dt.float16
    AF = mybir.ActivationFunctionType
    MUL = mybir.AluOpType.mult
    ADD = mybir.AluOpType.add
    ts = bass.ts

    nc = bacc.Bacc("TRN2", target_bir_lowering=False, debug=False,
                   enable_asserts=True, num_devices=1)

    # x_pre[p, co*T + t] = x[t, co*128+p]: per-partition contiguous lines
    xT = nc.dram_tensor("xT", [128, NCO * T], F16, kind="ExternalInput").ap()
    x32_d = nc.dram_tensor("x32", [32, T], F16, kind="ExternalInput").ap()
    cc_d = nc.dram_tensor("cc", [D, T], F16, kind="ExternalInput").ap()
    ss_d = nc.dram_tensor("ssw", [D, T], F16, kind="ExternalInput").ap()
    # veT2[p, ch*128 + dd] = 2*ve[128ch+p, head_slice+dd]
    veT_d = nc.dram_tensor("veT", [D, T], F16, kind="ExternalInput").ap()
    # w_pre[p, co*128 + d] = w[co*128+p, d]: contiguous per-partition lines
    wq_d = nc.dram_tensor("wq", [128, C], F16, kind="ExternalInput").ap()
    wk_d = nc.dram_tensor("wk", [128, C], F16, kind="ExternalInput").ap()
    wv_d = nc.dram_tensor("wv", [128, C], F16, kind="ExternalInput").ap()
    wp_d = nc.dram_tensor("wp", [D, C], F16, kind="ExternalInput").ap()
    wg_d = nc.dram_tensor("wg", [32, 1], F16, kind="ExternalInput").ap()
    # masks pre-arranged host-side to [p, m, i]
    mk_d = nc.dram_tensor("masks", [128, 8 * 512], F16, kind="ExternalInput").ap()
    on_d = nc.dram_tensor("ones", [128, 128], F16, kind="ExternalInput").ap()
    out_d = nc.dram_tensor("out", [T, C], F16, kind="ExternalOutput").ap()

    xT3 = xT.rearrange("p (co t) -> p co t", co=NCO)

    with tile.TileContext(nc) as tc:
        with tc.tile_pool(name="const", bufs=1) as cst:
            x32 = cst.tile([32, T], F16, tag="x32")
            w_sbs = []
            for wd, nm in ((wq_d, "wq"), (wk_d, "wk"), (wv_d, "wv")):
                w_sb = cst.tile([128, NCO, D], F16, tag=f"w{nm}")
                nc.sync.dma_start(w_sb[:], wd.rearrange("p (co d) -> p co d",
                                                        co=NCO))
                w_sbs.append(w_sb)
            wq_sb, wk_sb, wv_sb = w_sbs
            wg_sb = cst.tile([32, 1], F16, tag="wg")
            nc.sync.dma_start(wg_sb[:], wg_d)
            wp_sb = cst.tile([128, C], F16, tag="wp")
            mk_sb = cst.tile([128, 8, 512], F16, tag="mk")
            on_sb = cst.tile([128, 128], F16, tag="on")
            nc.sync.dma_start(on_sb[:], on_d)
            ident = cst.tile([128, 128], F16, tag="ident")
            make_identity(nc, ident[:])
            eps = cst.tile([128, 1], F32, tag="eps")
            nc.gpsimd.memset(eps[:], 1e-6)
            eb = cst.tile([128, 1], F32, tag="eb")
            nc.gpsimd.memset(eb[:], EXP_BIAS)
            gcol = cst.tile([128, 32], F16, tag="gcol")
            qTn = cst.tile([128, T], F16, tag="qTn")
            kTn = cst.tile([128, T], F16, tag="kTn")
            vsl = cst.tile([128, T // 128, D], F16, tag="vsl")
            cc_sb = cst.tile([128, T], F16, tag="cc")
            ss_sb = cst.tile([128, T], F16, tag="ssw")
            vet = cst.tile([128, 32, 128], F16, tag="ve")

            for _rep in range(nreps):
                # ---- phase 1: VE gate columns + q/k/v projections,
                # rope+rmsnorm, v transpose+gate. sumsq matmuls are emitted
                # after all three projections so the PE never stalls on the
                # DVE rope chain.
                with tc.tile_pool(name="xp", bufs=3) as xp, \
                     tc.tile_pool(name="sc1", bufs=4) as sc, \
                     tc.tile_pool(name="pps", bufs=3, space="PSUM") as pps, \
                     tc.tile_pool(name="gps", bufs=1, space="PSUM") as gps, \
                     tc.tile_pool(name="sqps", bufs=2, space="PSUM") as sqps, \
                     tc.tile_pool(name="tps", bufs=2, space="PSUM") as tps:
                    x_sb2 = None
                    for tb in range(NBLK):
                        sl = ts(tb, TB)
                        if tb % 2 == 0:
                            x_sb2 = xp.tile([128, NCO, 2 * TB], F16, tag="x")
                            vet3 = veT_d.rearrange("p (ch dd) -> p ch dd",
                                                   ch=32)
                            sl2 = ts(tb // 2, 2 * TB)
                            if tb == 0:
                                # split first chunk: block-0 projections
                                # start after a half-size transfer; the
                                # gate slice and rope tables slot between
                                nc.sync.dma_start(
                                    x_sb2[:, :, 0:TB], xT3[:, :, 0:TB])
                                if _rep == 0:
                                    nc.sync.dma_start(x32[:], x32_d)
                                    nc.sync.dma_start(cc_sb[:, sl2],
                                                      cc_d[:, sl2])
                                    nc.sync.dma_start(ss_sb[:, sl2],
                                                      ss_d[:, sl2])
                                nc.sync.dma_start(
                                    x_sb2[:, :, TB:2 * TB],
                                    xT3[:, :, TB:2 * TB])
                                if _rep == 0:
                                    nc.sync.dma_start(
                                        vet[:, 0:8, :], vet3[:, 0:8, :])
                            else:
                                nc.sync.dma_start(
                                    x_sb2[:], xT3[:, :, ts(tb // 2, 2 * TB)])
                                if _rep == 0:
                                    nc.sync.dma_start(cc_sb[:, sl2],
                                                      cc_d[:, sl2])
                                    nc.sync.dma_start(ss_sb[:, sl2],
                                                      ss_d[:, sl2])
                                    nc.sync.dma_start(
                                        vet[:, 4 * tb:4 * tb + 8, :],
                                        vet3[:, 4 * tb:4 * tb + 8, :])
                                    if tb == NBLK - 2:
                                        nc.sync.dma_start(wp_sb[:], wp_d)
                                        nc.sync.dma_start(
                                            mk_sb[:],
                                            mk_d.rearrange(
                                                "p (m i) -> p m i", m=8))
                        x_sb = x_sb2[:, :, ts(tb % 2, TB)]
                        sq_tiles = []
                        for w_sb in (wq_sb, wk_sb):
                            up = pps.tile([128, TB], F32, tag="proj")
                            for co in range(NCO):
                                nc.tensor.matmul(up[:], w_sb[:, co, :],
                                                 x_sb[:, co, :],
                                                 start=(co == 0), stop=(co == NCO - 1))
                            u16 = sc.tile([128, TB], F16, tag="u16")
                            nc.scalar.copy(u16[:], up[:])
                            # rope: p[d] = u[d]*ssw[d]; y = u*cc + swap64(p)
                            t1 = sc.tile([128, TB], F16, tag="t1")
                            nc.vector.tensor_tensor(t1[:], u16[:], cc_sb[:, sl], MUL)
                            p = sc.tile([128, TB], F16, tag="p")
                            nc.vector.tensor_tensor(p[:], u16[:], ss_sb[:, sl], MUL)
                            pr = sc.tile([128, TB], F16, tag="pr")
                            nc.vector.tensor_copy(pr[0:64, :], p[64:128, :])
                            nc.vector.tensor_copy(pr[64:128, :], p[0:64, :])
                            y = sc.tile([128, TB], F16, tag="y")
                            nc.vector.tensor_tensor(y[:], t1[:], pr[:], ADD)
                            sq = sc.tile([128, TB], F16, tag="sq")
                            nc.vector.tensor_tensor(sq[:], y[:], y[:], MUL)
                            sq_tiles.append((sq, y))
                        # v projection; gate applied post-transpose
                        vp = pps.tile([128, TB], F32, tag="proj")
                        for co in range(NCO):
                            nc.tensor.matmul(vp[:], wv_sb[:, co, :], x_sb[:, co, :],
                                             start=(co == 0), stop=(co == NCO - 1))
                        v16 = sc.tile([128, TB], F16, tag="v16")
                        nc.scalar.copy(v16[:], vp[:])
                        if tb == 0:
                            # gate cols: gcol[p,c] = sigmoid(x[128c+p,:32]@wg)
                            # emitted here so the PE has projection work
                            # first and the ACT sigmoid precedes all rsqrts
                            gp = gps.tile([128, 32], F32, tag="gps")
                            for cp in range(32):
                                nc.tensor.matmul(
                                    gp[:, cp:cp + 1],
                                    x32[:, ts(cp, 128)], wg_sb[:],
                                    start=True, stop=True)
                            nc.scalar.activation(gcol[:], gp[:], AF.Sigmoid)
                        # rms-norm scale (partition-axis sum via all-ones matmul)
                        for (sq, y), slab in zip(sq_tiles, (qTn, kTn)):
                            sp = sqps.tile([128, TB], F32, tag="sumsq")
                            nc.tensor.matmul(sp[:], on_sb[:], sq[:],
                                             start=True, stop=True)
                            rs = sc.tile([128, TB], F16, tag="rs")
                            nc.scalar.activation(rs[:], sp[:],
                                                 AF.Abs_reciprocal_sqrt,
                                                 scale=1.0 / D, bias=eps[:])
                            nc.vector.tensor_tensor(slab[:, sl], y[:], rs[:], MUL)
                        # v^T chunks + VE gate: vsl = (veT*g) + v^T
                        for kk in range(4):
                            ch = 4 * tb + kk
                            tp = tps.tile([128, 128], F16, tag="tp")
                            nc.tensor.transpose(tp[:], v16[:, ts(kk, 128)], ident[:])
                            nc.vector.scalar_tensor_tensor(
                                vsl[:, ch, :], vet[:, ch, :],
                                gcol[:, ch:ch + 1], tp[:], MUL, ADD)

                # ---- phase 2: windowed attention + output projection ----
                # ACT funcs here: Exp only. Chunk pairs share one psum tile
                # and one exp. The S matmuls for pair p+LAG are emitted
                # before the PV of pair p so the PE never stalls on the ACT
                # exp chain.
                with tc.tile_pool(name="ptp", bufs=6) as ptp, \
                     tc.tile_pool(name="acp", bufs=2) as acp, \
                     tc.tile_pool(name="sc2", bufs=4) as sc2, \
                     tc.tile_pool(name="outp", bufs=4) as outp, \
                     tc.tile_pool(name="sps", bufs=2, space="PSUM") as sps, \
                     tc.tile_pool(name="yps", bufs=2, space="PSUM") as yps, \
                     tc.tile_pool(name="dps", bufs=1, space="PSUM") as dps, \
                     tc.tile_pool(name="ops", bufs=1, space="PSUM") as ops:
                    # dummy exp on a tiny tile: forces the exp table load
                    # onto the ACT queue early, overlapping phase-1 tail
                    warm = sc2.tile([128, 1], F16, tag="warm")
                    nc.scalar.activation(warm[:], eb[:], AF.Exp)

                    def emit_outproj(yt, rc_col, i0, tcc):
                        # softmax 1/denominator rides the psum drain as a
                        # per-partition (query-row) scale
                        ost = outp.tile([128, 1024], F16, tag="ost")
                        for hh in range(2):
                            op2 = ops.tile([128, 512], F32, tag="opair")
                            nc.tensor.matmul(op2[:],
                                             yt[:, ts(tcc, 128)],
                                             wp_sb[:, ts(hh, 512)],
                                             start=True, stop=True)
                            if hh == 0:
                                nc.scalar.activation(
                                    ost[:, ts(hh, 512)], op2[:], AF.Copy,
                                    scale=rc_col[:, tcc:tcc + 1])
                            else:
                                nc.vector.tensor_scalar(
                                    ost[:, ts(hh, 512)], op2[:],
                                    rc_col[:, tcc:tcc + 1], None, MUL)
                        nc.sync.dma_start(
                            out_d[i0 + 128 * tcc:i0 + 128 * (tcc + 1), :],
                            ost[:])

                    def emit_tail(yp, acc):
                        """column denominator -> 1/d columns -> y16 drain"""
                        dp = dps.tile([128, 4], F32, tag="d")
                        for c in range(4):
                            nc.tensor.matmul(dp[:, c:c + 1],
                                             acc[:, ts(c, 128)],
                                             on_sb[:, 0:1],
                                             start=True, stop=True)
                        rc_col = sc2.tile([128, 4], F32, tag="rc")
                        nc.vector.reciprocal_approx_fast(rc_col[:], dp[:])
                        y16 = sc2.tile([128, TB], F16, tag="y16")
                        nc.scalar.copy(y16[:], yp[:])
                        return y16, rc_col

                    LAG = 1   # pairs of lookahead between exp and PV
                    prev = None   # (yp, acc, i0) of previous block
                    pending_out = None
                    for b in range(NBLK):
                        i0 = TB * b
                        pairs = _pair_list(b)
                        n = len(pairs)
                        yp = yps.tile([128, TB], F32, tag="y")
                        acc = acp.tile([128, TB], F16, tag="acc")
                        pts = {}
                        nch = sum(len(pr) for pr in pairs)
                        ich = 0   # chunk counter for PV start/stop flags
                        for step in range(n + LAG):
                            if step < n:
                                pair = pairs[step]
                                wtot = sum(hi - lo for (_, _, lo, hi, _) in pair)
                                sp2 = sps.tile([128, 1024], F32, tag="spair")
                                for (j0, mi, lo, hi, off) in pair:
                                    nc.tensor.matmul(
                                        sp2[:, off:off + hi - lo],
                                        kTn[:, j0:j0 + 128],
                                        qTn[:, i0 + lo:i0 + hi],
                                        start=True, stop=True)
                                pt = ptp.tile([128, 1024], F16, tag="pt")
                                nc.scalar.activation(pt[:, 0:wtot],
                                                     sp2[:, 0:wtot],
                                                     AF.Exp, scale=SCALE,
                                                     bias=eb[:])
                                for (j0, mi, lo, hi, off) in pair:
                                    if mi is not None:
                                        # masks on the (otherwise idle)
                                        # Pool/GpSimd engine
                                        mlo = 128 * (mi if mi < 4 else mi - 4)
                                        psl = pt[:, off + mlo - lo:
                                                 off + mlo - lo + 128]
                                        nc.gpsimd.tensor_tensor(
                                            psl, psl,
                                            mk_sb[:, mi, mlo:mlo + 128], MUL)
                                # denominator fold (f16; <=12 summands/elem)
                                for (j0, mi, lo, hi, off) in pair:
                                    src = pt[:, off:off + hi - lo]
                                    if step == 0 and off == 0:
                                        nc.vector.tensor_copy(acc[:], src)
                                    else:
                                        nc.vector.tensor_tensor(
                                            acc[:, lo:hi], acc[:, lo:hi],
                                            src, ADD)
                                pts[step] = pt
                            if step == 1 and prev is not None:
                                pyp, pacc, pi0 = prev
                                py16, prc = emit_tail(pyp, pacc)
                                pending_out = (py16, prc, pi0)
                                prev = None
                            if step == 3 and pending_out is not None:
                                for tcc in range(4):
                                    emit_outproj(*pending_out, tcc)
                                pending_out = None
                            idx = step - LAG
                            if 0 <= idx < n:
                                pt = pts.pop(idx)
                                for (j0, mi, lo, hi, off) in pairs[idx]:
                                    st, sp_ = (ich == 0), (ich == nch - 1)
                                    nc.tensor.matmul(
                                        yp[:, lo:hi],
                                        vsl[:, j0 // 128, :],
                                        pt[:, off:off + hi - lo],
                                        start=st, stop=sp_)
                                    ich += 1
                        prev = (yp, acc, i0)
                    # final block tail + outproj
                    py16, prc = emit_tail(prev[0], prev[1])
                    for tcc in range(4):
                        emit_outproj(py16, prc, prev[2], tcc)

    nc.finalize()
    return nc


def _w_pre(w):
    # w_pre[p, co*128 + d] = w[co*128+p, d]
    return np.ascontiguousarray(
        w.reshape(NCO, 128, D).transpose(1, 0, 2).reshape(128, C)
    ).astype(np.float16)


def _build_masks():
    jj = np.arange(128)[:, None]
    ii = np.arange(512)[None, :]
    mk = np.zeros((8, 128, 512), dtype=np.float16)
    for m in range(4):
        mk[m] = (ii < jj + 128 * m).astype(np.float16)
    for m in range(4):
        mk[4 + m] = (ii >= jj + 128 * m).astype(np.float16)
    # host pre-arrange to [p, m, i] so the DMA is a straight copy
    return np.ascontiguousarray(mk.transpose(1, 0, 2).reshape(128, 8 * 512))


def kernel(x, ve, cos, sin, wq, wk, wv, w_gate, w_proj, window_size):
    from concourse.bass_utils import run_bass_kernel_spmd

    assert int(np.asarray(window_size)) == WIN
    x = np.asarray(x, dtype=np.float32)
    ve = np.asarray(ve, dtype=np.float32)
    cos = np.asarray(cos, dtype=np.float32).reshape(T, 64)
    sin = np.asarray(sin, dtype=np.float32).reshape(T, 64)
    wq = np.asarray(wq, dtype=np.float32)
    wk = np.asarray(wk, dtype=np.float32)
    wv = np.asarray(wv, dtype=np.float32)
    w_gate = np.asarray(w_gate, dtype=np.float32)
    w_proj = np.asarray(w_proj, dtype=np.float32)
    assert x.shape == (1, T, C) and ve.shape == (1, T, C)

    if "nc" not in _prog_cache:
        _prog_cache["nc"] = _build_program()
    nc = _prog_cache["nc"]

    # x_pre[p, co*T + t] = x[t, co*128+p]
    xT_h = np.ascontiguousarray(
        x[0].T.reshape(NCO, 128, T).transpose(1, 0, 2).reshape(128, NCO * T)
    ).astype(np.float16)
    x32_h = np.ascontiguousarray(x[0][:, 0:32].T).astype(np.float16)
    cosT, sinT = cos.T, sin.T                                # [64, T]
    cc = np.concatenate([cosT, cosT], axis=0).astype(np.float16)
    # p[d] = u[d]*ssw[d]; y[d] = u[d]*cc[d] + p[swap(d)]
    # => ssw = [-sinT; sinT]
    ssw = np.concatenate([-sinT, sinT], axis=0).astype(np.float16)
    masks = _build_masks()

    in_maps = []
    for h in range(NCORES):
        d = D * h
        # veT2[p, 128ch+dd] = 2*ve[128ch+p, d+dd]
        ve2 = np.ascontiguousarray(
            (2.0 * ve[0][:, d:d + D]).reshape(32, 128, 128)
            .transpose(1, 0, 2).reshape(128, T)).astype(np.float16)
        in_maps.append({
            "xT": xT_h,
            "x32": x32_h,
            "cc": cc,
            "ssw": ssw,
            "veT": ve2,
            "wq": _w_pre(wq[:, d:d + D]),
            "wk": _w_pre(wk[:, d:d + D]),
            "wv": _w_pre(wv[:, d:d + D]),
            "wp": np.ascontiguousarray(w_proj[d:d + D, :]).astype(np.float16),
            "wg": w_gate[:, h:h + 1].astype(np.float16),
            "masks": masks,
            "ones": np.ones((128, 128), dtype=np.float16),
        })

    global _last_in_maps
    _last_in_maps = in_maps
    res = run_bass_kernel_spmd(nc, in_maps, core_ids=list(range(NCORES)))
    out = np.zeros((T, C), dtype=np.float32)
    for h in range(NCORES):
        out += res.results[h]["out"].astype(np.float32)
    return out.reshape(1, T, C)
